# revision 23
# baseline (speedup 1.0000x reference)
"""Adaptive-threshold LIF neuron recurrence (SNN) on 8 Trainium2 NeuronCores.

Strategy
--------
The recurrence is data-parallel over the 262144 neurons except for a scalar
firing-rate EMA that couples every neuron at every timestep (the spike MEAN
feeds the next step's threshold).  A per-step AllReduce would cost ~10us x
128 steps, so instead:

  host:   estimate the threshold-offset sequence C0[t] from a neuron
          subsample (cheap numpy sim)
  device: ONE data-parallel pass with C0 per core, recording per-step
          spike/membrane/adaptation partial sums (exact f32 integers for
          spikes) AND a per-neuron count of "near-margin" steps
          (|u - C0| <= theta0)
  host:   compute the exact global EMA chain from the (integer) spike sums
          -> corrected C[t]; every neuron with no near-margin step provably
          has an identical trajectory under the corrected C; the few
          remaining neurons are re-simulated in numpy with bit-exact
          replicas of the device arithmetic, iterating the tiny fixed point
          until the C chain is stable; spikes and sums are patched.

To minimize per-step work on the (bottleneck) Vector engine, the state is
kept in geometrically rescaled coordinates with compile-time per-step
scale immediates:

  x'_t   = x_t / beta^t                (host prescale)
  S_t    = S_{t-1} + x'_t              (GPSIMD add; == syn_t/beta^t)
  synv_t = (beta^t/alpha^t) * S_t      (ACT copy; == syn_t/alpha^t)
  m_t    = m'_{t-1} + synv_t           (GPSIMD add; m == mem/alpha^t)
  u_t    = k1*m_t - W_{t-1}            (V stt, accum; u == (mem-.1*adapt)/a^t)
  spike  = u_t >= C'_t                 (V tensor_scalar, accum; C' = C/a^t)
  e_t    = relu(u_t - C'_t)            (ACT relu, bias=-C')
  m'_t   = spike ? e_t : m_t           (V copy_predicated)
  tmp    = (kk/alpha)*u_t              (ACT copy)
  W_t    = (g2/alpha)*W_{t-1} + tmp    (V stt, accum; W == 0.1*gamma*adapt
                                        / alpha^{t+1})
  ab     = |u_t - C'_t|                (ACT abs)
  sgn    = sign(ab - theta0/alpha^t)   (ACT sign; -1/0 when near margin)
  asum  += sgn                         (GPSIMD add; == T iff provably safe)

Spike sums are sums of 0/1 values, so every f32 reduction of them is exact
(integers < 2^24), which makes the host EMA chain exact.  A 2-pass fully
on-device variant (pass1 -> AllReduce -> pass2) is kept as a fallback.

Per-core layout: features sharded 8 ways (1024 per core), time-major tiles
x[tb, p, g*256+j] = x'[t = tb*G+g, neuron n = p*256+j], n = b*1024+f.
"""

import math
import sys
import time

if "/opt/trn_rl_repo" not in sys.path:
    sys.path.insert(0, "/opt/trn_rl_repo")

import numpy as np

# ---------------------------------------------------------------- constants
B, F, T = 32, 8192, 128
N_CORES = 8
F_LOC = F // N_CORES            # 1024 features per core
N_LOC = B * F_LOC               # 32768 neurons per core
N_GLOB = B * F                  # 262144 neurons total
P = 128                         # SBUF partitions
FD = N_LOC // P                 # 256 free-dim elems per step tile
G = 16                          # timesteps per DMA group
TB = T // G                     # 8 groups

# exact f32 constants as produced by the jax reference (verified bitwise)
ALPHA = np.uint32(0x3F7383C5).view(np.float32)  # exp(-.001/.02)
BETA = np.uint32(0x3F519857).view(np.float32)   # exp(-.001/.005)
GAMMA = np.uint32(0x3F7D73E8).view(np.float32)  # exp(-.001/.1)
C1G = np.uint32(0x3C230600).view(np.float32)    # 1 - gamma
ADAPT_STRENGTH = np.float32(0.1)
HOMEO_RATE = np.float32(0.01)
THRESHOLD_BASE = np.float32(1.0)

K1 = np.float32(1.0 - 0.1 * float(C1G))            # 1 - 0.1*(1-gamma)
K2 = np.float32(0.1 * float(GAMMA) * float(C1G))   # 0.1*gamma*(1-gamma)
# adaptation state W ~ 0.1*gamma*adapt, updated from u:
#   G_t = g2*G_{t-1} + kk*u_t
KK = np.float32(float(K2) / float(K1))
G2 = np.float32(float(GAMMA) + float(K2) / float(K1))

# per-step geometric scales
A64 = np.float64(ALPHA) ** np.arange(T + 1)      # alpha^t
B64 = np.float64(BETA) ** np.arange(T)           # beta^t
IB64 = 1.0 / B64                                 # input prescale
SV32 = (B64 / A64[:T]).astype(np.float32)        # synv scale immediates
KKA = np.float32(float(KK) / float(ALPHA))
G2A = np.float32(float(G2) / float(ALPHA))

N_PASSES = 1          # 1 = single pass + host margin correction (default)
MARGIN_CAP = 40000    # fall back to a device re-run above this many neurons
THETA0 = 3e-4         # margin radius tracked on device (unscaled u units)

_BUILD_CACHE = {}


# ------------------------------------------------------------- device build
def _build_kernel(n_passes=N_PASSES, base=1.0, bias=0.0):
    key = (n_passes, float(base), float(bias))
    if key in _BUILD_CACHE:
        return _BUILD_CACHE[key]

    import concourse.bacc as bacc
    import concourse.mybir as mybir
    from concourse import tile

    DT = mybir.dt.float32
    AF = mybir.ActivationFunctionType
    OP = mybir.AluOpType
    I32 = mybir.dt.int32

    nc = bacc.Bacc(None, target_bir_lowering=False, debug=False,
                   num_devices=N_CORES)

    x_in = nc.dram_tensor("x", [TB, P, G * FD], DT, kind="ExternalInput")
    # rows: 0 = C' (scaled C), 1 = -C', 2 = -theta0/alpha^t, 3 = 1/alpha^t
    c0_in = nc.dram_tensor("c0", [4, T], DT, kind="ExternalInput")

    spk_out = nc.dram_tensor("spk", [TB, P, G * FD], DT, kind="ExternalOutput")
    acc_out = nc.dram_tensor("acc", [3, P, T], DT, kind="ExternalOutput")
    crow_out = nc.dram_tensor("crow", [max(n_passes, 2), T], DT,
                              kind="ExternalOutput")
    xtra_out = nc.dram_tensor("xtra", [1, 8], DT, kind="ExternalOutput")
    mm_out = nc.dram_tensor("mm", [P, FD], DT, kind="ExternalOutput")

    with tile.TileContext(nc) as tc:
        with (
            tc.tile_pool(name="state", bufs=1) as st,
            tc.tile_pool(name="xload", bufs=3) as xl,
            tc.tile_pool(name="sout", bufs=2) as so,
            tc.tile_pool(name="psum", bufs=2, space="PSUM") as ps,
            tc.tile_pool(name="dram", bufs=1, space="DRAM") as dram,
        ):
            mem = st.tile([P, FD], DT, tag="mem")
            gst = st.tile([P, FD], DT, tag="gst")
            mm = st.tile([P, FD], DT, tag="mm")
            spk_s = st.tile([P, FD], DT, tag="spk_s")
            ones = st.tile([P, 1], DT, tag="ones")
            nc.vector.memset(ones[:], 1.0)
            track_margin = n_passes == 1
            if track_margin:
                nc.gpsimd.memset(mm[:], 0.0)

            # broadcast C rows ([128, T]): positive scaled C for the spike
            # compare, negated scaled C for ACT bias, scaled margin radius
            cmat = [
                st.tile([P, T], DT, tag=f"cmat{p}", name=f"cmat{p}")
                for p in range(n_passes)
            ]
            nmat = [
                st.tile([P, T], DT, tag=f"nmat{p}", name=f"nmat{p}")
                for p in range(n_passes)
            ]
            thmat = st.tile([P, T], DT, tag="thmat")
            c0row = st.tile([1, T], DT, tag="c0row")
            ncrow = st.tile([1, T], DT, tag="ncrow")
            throw = st.tile([1, T], DT, tag="throw")
            iarow = st.tile([1, T], DT, tag="iarow")
            nc.sync.dma_start(c0row[:], c0_in[0:1, :])
            nc.sync.dma_start(ncrow[:], c0_in[1:2, :])
            nc.sync.dma_start(throw[:], c0_in[2:3, :])
            nc.sync.dma_start(iarow[:], c0_in[3:4, :])
            nc.gpsimd.partition_broadcast(cmat[0][:], c0row[:], channels=P)
            nc.gpsimd.partition_broadcast(nmat[0][:], ncrow[:], channels=P)
            nc.gpsimd.partition_broadcast(thmat[:], throw[:], channels=P)

            # accumulators: [128, T]: spike counts, u sums, W sums
            acc_spk = st.tile([P, T], DT, tag="acc_spk")
            acc_u = st.tile([P, T], DT, tag="acc_u")
            acc_w = st.tile([P, T], DT, tag="acc_w")

            # constant row of 0.99 for the ema scan (2-pass mode)
            row99 = st.tile([1, T], DT, tag="row99")
            nc.vector.memset(row99[:], 0.99)

            for p_i in range(n_passes):
                last = p_i == n_passes - 1
                C = cmat[p_i]
                NC = nmat[p_i]
                syn = st.tile([P, FD], DT, tag="syn", name="syn0", bufs=2)
                nc.gpsimd.memset(syn[:], 0.0)
                nc.gpsimd.memset(mem[:], 0.0)
                nc.vector.memset(gst[:], 0.0)

                for tb in range(TB):
                    if tb == 0:
                        # split the first load 4+2 ways so compute starts
                        # as soon as the first column chunk lands
                        xt_a = xl.tile([P, 4 * FD], DT, tag="xta",
                                       name="xta")
                        for q in range(4):
                            nc.sync.dma_start(
                                xt_a[:, q * FD:(q + 1) * FD],
                                x_in[0, :, q * FD:(q + 1) * FD])
                        xt_b = xl.tile([P, 12 * FD], DT, tag="xtb",
                                       name="xtb")
                        nc.sync.dma_start(xt_b[:, 0:6 * FD],
                                          x_in[0, :, 4 * FD:10 * FD])
                        nc.sync.dma_start(xt_b[:, 6 * FD:],
                                          x_in[0, :, 10 * FD:])
                    else:
                        xt = xl.tile([P, G * FD], DT, tag="xt", name="xt")
                        nc.sync.dma_start(xt[:], x_in[tb, :, :])
                    if last:
                        ot = so.tile([P, G * FD], DT, tag="ot")
                    for g in range(G):
                        t = tb * G + g
                        if tb == 0:
                            if g < 4:
                                xs = xt_a[:, g * FD:(g + 1) * FD]
                            else:
                                xs = xt_b[:, (g - 4) * FD:(g - 3) * FD]
                        else:
                            xs = xt[:, g * FD:(g + 1) * FD]
                        u = st.tile([P, FD], DT, tag="u", name="u", bufs=2)
                        e = st.tile([P, FD], DT, tag="e", name="e", bufs=2)
                        tmp = st.tile([P, FD], DT, tag="tmp", name="tmp",
                                      bufs=2)
                        ab = st.tile([P, FD], DT, tag="ab", name="ab",
                                     bufs=2)
                        sg = st.tile([P, FD], DT, tag="sg", name="sg",
                                     bufs=2)
                        # S_t = S_{t-1} + x'_t (GPSIMD, double-buffered)
                        syn_new = st.tile([P, FD], DT, tag="syn",
                                          name="syn", bufs=2)
                        nc.gpsimd.tensor_tensor(syn_new[:], syn[:], xs,
                                                OP.add)
                        syn = syn_new
                        # synv_t = (beta^t/alpha^t) * S_t (ACT)
                        synu = st.tile([P, FD], DT, tag="synu", name="synu",
                                       bufs=2)
                        nc.scalar.activation(synu[:], syn[:], AF.Copy,
                                             scale=float(SV32[t]))
                        # m_t = m'_{t-1} + synv_t (GPSIMD)
                        nc.gpsimd.tensor_tensor(mem[:], mem[:], synu[:],
                                                OP.add)
                        # u_t = k1*m - W (V stt, accum -> u sums)
                        nc.vector.scalar_tensor_tensor(
                            u[:], mem[:], float(K1), gst[:],
                            OP.mult, OP.subtract,
                            accum_out=acc_u[:, t:t + 1] if last else None,
                        )
                        # reset value e = relu(u + (-C')) (ACT; on the
                        # critical V loop so emitted early)
                        nc.scalar.activation(
                            e[:], u[:], AF.Relu, bias=NC[:, t:t + 1],
                            scale=1.0,
                        )
                        if last:
                            sp_ap = ot[:, g * FD:(g + 1) * FD]
                        else:
                            sp_ap = spk_s[:]
                        # spike = (u >= C'), accum(reduce add) -> step sums
                        nc.vector.tensor_scalar(
                            sp_ap, u[:], C[:, t:t + 1], 0.0,
                            OP.is_ge, OP.add,
                            accum_out=acc_spk[:, t:t + 1],
                        )
                        nc.vector.copy_predicated(
                            mem[:], sp_ap.bitcast(I32), e[:]
                        )
                        # W_t = (g2/a)*W + (kk/a)*u (ACT + V stt, accum)
                        nc.scalar.activation(tmp[:], u[:], AF.Copy,
                                             scale=float(KKA))
                        nc.vector.scalar_tensor_tensor(
                            gst[:], gst[:], float(G2A), tmp[:],
                            OP.mult, OP.add,
                            accum_out=acc_w[:, t:t + 1] if last else None,
                        )
                        if track_margin:
                            nc.scalar.activation(
                                ab[:], u[:], AF.Abs, bias=NC[:, t:t + 1],
                                scale=1.0,
                            )
                            nc.scalar.activation(
                                sg[:], ab[:], AF.Sign,
                                bias=thmat[:, t:t + 1], scale=1.0,
                            )
                            nc.gpsimd.tensor_tensor(mm[:], mm[:], sg[:],
                                                    OP.add)
                    if last:
                        nc.sync.dma_start(spk_out[tb, :, :], ot[:])

                # ---- inter-pass: global spike sums -> next C (2-pass mode)
                if not last:
                    pt = ps.tile([1, T], DT, tag="pt")
                    nc.tensor.matmul(pt[:], ones[:], acc_spk[:],
                                     start=True, stop=True)
                    srow = st.tile([1, T], DT, tag=f"srow{p_i}",
                                   name=f"srow{p_i}")
                    nc.scalar.copy(srow[:], pt[:])
                    cc_in = dram.tile([1, T], DT, name=f"ccin{p_i}")
                    cc_out = dram.tile([1, T], DT, name=f"ccout{p_i}")
                    nc.sync.dma_start(cc_in[:], srow[:])
                    nc.gpsimd.collective_compute(
                        "AllReduce", OP.add,
                        replica_groups=[list(range(N_CORES))],
                        ins=[cc_in.opt()], outs=[cc_out.opt()],
                    )
                    sglob = st.tile([1, T], DT, tag=f"sglob{p_i}",
                                    name=f"sglob{p_i}")
                    nc.sync.dma_start(sglob[:], cc_out[:])
                    # ema chain: msc = sums * (0.01/N_GLOB); scan; homeo; C
                    mean_scale = np.float32(
                        np.float32(0.01) * np.float32(1.0 / N_GLOB))
                    msc = st.tile([1, T], DT, tag=f"msc{p_i}",
                                  name=f"msc{p_i}")
                    nc.vector.tensor_scalar(
                        msc[:], sglob[:], float(mean_scale), None, OP.mult)
                    ema = st.tile([1, T], DT, tag=f"ema{p_i}",
                                  name=f"ema{p_i}")
                    nc.vector.tensor_tensor_scan(
                        ema[:], row99[:], msc[:], 0.0, OP.mult, OP.add)
                    hm = st.tile([1, T], DT, tag=f"hm{p_i}", name=f"hm{p_i}")
                    nc.vector.tensor_scalar(
                        hm[:], ema[:], float(HOMEO_RATE), 0.0,
                        OP.subtract, OP.max)
                    cn = st.tile([1, T], DT, tag=f"cn{p_i}", name=f"cn{p_i}")
                    # C[0] is pass-independent (ema starts at 0; scale=1)
                    nc.vector.tensor_copy(cn[:, 0:1], c0row[:, 0:1])
                    nc.vector.tensor_scalar(
                        cn[:, 1:T], hm[:, 0:T - 1], float(ADAPT_STRENGTH),
                        float(base), OP.mult, OP.add)
                    if float(bias) != 0.0:
                        nc.vector.tensor_scalar(
                            cn[:, 1:T], cn[:, 1:T], float(bias), None,
                            OP.add)
                    # scale to C' = C/alpha^t (input row 3 = 1/alpha^t)
                    csc = st.tile([1, T], DT, tag=f"csc{p_i}",
                                  name=f"csc{p_i}")
                    nc.vector.tensor_tensor(csc[:], cn[:], iarow[:],
                                            OP.mult)
                    ncn = st.tile([1, T], DT, tag=f"ncn{p_i}",
                                  name=f"ncn{p_i}")
                    nc.vector.tensor_scalar(
                        ncn[:], csc[:], -1.0, None, OP.mult)
                    nc.gpsimd.partition_broadcast(
                        cmat[p_i + 1][:], csc[:], channels=P)
                    nc.gpsimd.partition_broadcast(
                        nmat[p_i + 1][:], ncn[:], channels=P)
                    nc.sync.dma_start(crow_out[p_i + 1:p_i + 2, :], cn[:])

            # ---- final extras: sum of post-reset scaled mem at t=T-1
            memsum = st.tile([P, 1], DT, tag="memsum")
            nc.vector.reduce_sum(memsum[:], mem[:], axis=mybir.AxisListType.X)
            ptx = ps.tile([1, 1], DT, tag="ptx")
            nc.tensor.matmul(ptx[:], ones[:], memsum[:], start=True, stop=True)
            xrow = st.tile([1, 8], DT, tag="xrow")
            nc.vector.memset(xrow[:], 0.0)
            nc.scalar.copy(xrow[:, 0:1], ptx[:])
            nc.sync.dma_start(xtra_out[:, :], xrow[:])

            # C' used in pass 0 (echo back for debugging)
            nc.sync.dma_start(crow_out[0:1, :], c0row[:, :])

            # accumulators + margin counters out
            nc.sync.dma_start(acc_out[0, :, :], acc_spk[:])
            nc.sync.dma_start(acc_out[1, :, :], acc_u[:])
            nc.sync.dma_start(acc_out[2, :, :], acc_w[:])
            if track_margin:
                nc.sync.dma_start(mm_out[:, :], mm[:])
            else:
                nc.sync.dma_start(mm_out[:, :], spk_s[:])

    nc.compile()
    _BUILD_CACHE[key] = nc
    return nc


# --------------------------------------------------------------- host logic
def _ema_chain_from_sums(sums, base, bias):
    """Replicate the reference's scalar f32 EMA/homeo chain from global
    per-step spike sums (exact integers).  Returns (C[t] used AT step t,
    final ema)."""
    ema = np.float32(0.0)
    C = np.zeros(T, np.float32)
    for t in range(T):
        homeo = np.float32(ADAPT_STRENGTH * max(np.float32(ema - HOMEO_RATE),
                                                np.float32(0.0)))
        C[t] = np.float32(np.float32(base + homeo) + bias)
        mean = np.float32(np.float32(sums[t]) * np.float32(1.0 / N_GLOB))
        ema = np.float32(math.fma(float(np.float32(0.99)), float(ema),
                                  float(np.float32(np.float32(0.01) * mean))))
    return C, ema


def _estimate_c0(input_current, base, bias, stride=16):
    """Cheap numpy f32 simulation of a strided neuron subsample with a local
    EMA to estimate the C[t] threshold-offset sequence."""
    x = np.ascontiguousarray(
        input_current[:, ::stride, :], dtype=np.float32)  # [B, F/stride, T]
    nb, nf, nt = x.shape
    x = x.reshape(nb * nf, nt)
    syn = np.zeros(nb * nf, np.float32)
    mem = np.zeros(nb * nf, np.float32)
    adp = np.zeros(nb * nf, np.float32)
    ema = np.float32(0.0)
    C = np.zeros(nt, np.float32)
    for t in range(nt):
        syn = BETA * syn + x[:, t]
        mem = ALPHA * mem + syn
        homeo = np.float32(ADAPT_STRENGTH * max(np.float32(ema - HOMEO_RATE),
                                                np.float32(0.0)))
        adp = GAMMA * adp + C1G * mem
        C[t] = np.float32(np.float32(base + homeo) + bias)
        thr = C[t] + ADAPT_STRENGTH * adp
        spike = (mem >= thr)
        ema = np.float32(0.99 * ema + 0.01 * np.float32(spike.mean()))
        mem = np.where(spike, mem - thr, mem).astype(np.float32)
    return C


def _chat_rows(C):
    """Device input rows for a given (unscaled) C chain: [C', -C',
    -theta0/alpha^t, 1/alpha^t], all f32 [4, T]."""
    chat = (C.astype(np.float64) / A64[:T]).astype(np.float32)
    th = (np.float64(THETA0) / A64[:T]).astype(np.float32)
    ia = (1.0 / A64[:T]).astype(np.float32)
    return np.stack([chat, -chat, -th, ia]).astype(np.float32), chat


def _sim_neurons(x_rows, chat):
    """Bit-exact numpy replica of the device per-neuron chain (scaled
    coordinates).  x_rows: [n, T] f32 prescaled inputs.  chat: [T] f32
    scaled C' sequence.  Returns per-step spikes/u/W plus final m'."""
    n = x_rows.shape[0]
    syn = np.zeros(n, np.float32)
    mem = np.zeros(n, np.float32)
    gst = np.zeros(n, np.float32)
    spikes = np.zeros((n, T), np.float32)
    u_tr = np.zeros((n, T), np.float32)
    w_tr = np.zeros((n, T), np.float32)
    for t in range(T):
        syn = (syn + x_rows[:, t]).astype(np.float32)
        synv = (syn * SV32[t]).astype(np.float32)
        mem = (mem + synv).astype(np.float32)
        u = ((mem * K1).astype(np.float32) - gst).astype(np.float32)
        u_tr[:, t] = u
        sp = u >= chat[t]
        spikes[:, t] = sp
        ev = np.maximum((u - chat[t]).astype(np.float32), np.float32(0.0))
        mem = np.where(sp, ev, mem).astype(np.float32)
        tmp = (u * KKA).astype(np.float32)
        gst = ((gst * G2A).astype(np.float32) + tmp).astype(np.float32)
        w_tr[:, t] = gst
    return {"spikes": spikes, "u": u_tr, "w": w_tr, "m_last": mem}


def _prescale(input_current):
    """x'_[b,f,t] = f32(x / beta^t) plus its per-step global f64 sums."""
    x64 = input_current.astype(np.float64) * IB64[None, None, :]
    xp = x64.astype(np.float32)
    sxp = x64.sum(axis=(0, 1))
    return xp, sxp


def _shard_input(xp):
    """prescaled [B, F, T] -> per-core [TB, P, G*FD] time-major tiles."""
    xs = []
    for k in range(N_CORES):
        xk = xp[:, k * F_LOC:(k + 1) * F_LOC, :]  # [B, F_LOC, T]
        xk = np.ascontiguousarray(np.moveaxis(xk, -1, 0)).reshape(T, N_LOC)
        xk = xk.reshape(TB, G, P, FD).transpose(0, 2, 1, 3)
        xs.append(np.ascontiguousarray(xk.reshape(TB, P, G * FD),
                                       dtype=np.float32))
    return xs


def _unshard_spikes(spk_cores):
    """per-core [TB, P, G*FD] -> [B, F, T] float32."""
    out = np.empty((B, F, T), np.float32)
    for k, sk in enumerate(spk_cores):
        sk = sk.reshape(TB, P, G, FD).transpose(0, 2, 1, 3).reshape(T, N_LOC)
        sk = np.moveaxis(sk.reshape(T, B, F_LOC), 0, -1)
        out[:, k * F_LOC:(k + 1) * F_LOC, :] = sk
    return out


def _run_device(xs, crows, base, bias, n_passes):
    from concourse.bass_utils import run_bass_kernel_spmd

    nc = _build_kernel(n_passes, float(base), float(bias))
    in_maps = [{"x": xs[k], "c0": crows} for k in range(N_CORES)]
    res = run_bass_kernel_spmd(nc, in_maps, core_ids=list(range(N_CORES)))
    return res.results


def _unscale_sums(s_u, s_w, xtra_sum):
    """Scaled per-step u/W sums -> unscaled mem_pre / G sums (f64)."""
    s_w_prev = np.concatenate([[0.0], s_w[:-1]])
    s_mem = A64[:T] * (s_u + s_w_prev) / np.float64(K1)
    s_g = A64[1:T + 1] * s_w
    mlast = xtra_sum * A64[T - 1]
    return s_mem, s_g, mlast


def kernel(input_current, threshold_scale, adaptation_bias):
    input_current = np.asarray(input_current, np.float32)
    threshold_scale = np.asarray(threshold_scale, np.float32)
    adaptation_bias = np.asarray(adaptation_bias, np.float32)
    assert input_current.shape == (B, F, T)

    scale_v = np.float32(threshold_scale.reshape(-1)[0])
    bias_v = np.float32(adaptation_bias.reshape(-1)[0])
    base_v = np.float32(THRESHOLD_BASE * scale_v)

    c0 = _estimate_c0(input_current, base_v, bias_v)
    xp, sxp = _prescale(input_current)
    xs = _shard_input(xp)

    if N_PASSES != 1:
        return _kernel_multipass(xs, sxp, c0, base_v, bias_v)

    for attempt in range(3):
        crows, chat0 = _chat_rows(c0)
        r = _run_device(xs, crows, base_v, bias_v, 1)
        out = _finish_single(xp, sxp, c0, chat0, base_v, bias_v, r)
        if out is not None:
            return out
        # margin set too large -> c0 was far off; retry with corrected C
        acc = np.stack([rk["acc"] for rk in r])
        s_spk = acc.astype(np.float64).sum(axis=(0, 2))[0]
        c0, _ = _ema_chain_from_sums(s_spk, base_v, bias_v)
    # last resort: fully on-device 2-pass mode
    return _kernel_multipass(xs, sxp, c0, base_v, bias_v)


def _finish_single(xp, sxp, c0, chat0, base_v, bias_v, r):
    acc = np.stack([rk["acc"] for rk in r])       # [8, 3, P, T]
    g_sums = acc.astype(np.float64).sum(axis=(0, 2))
    s_spk, s_u, s_w = g_sums
    mm = np.stack([rk["mm"] for rk in r])          # [8, P, FD]
    mm_flat = mm.reshape(N_CORES, N_LOC)           # neuron n = p*FD + j

    # candidate corrected C chain
    C1, _ = _ema_chain_from_sums(s_spk, base_v, bias_v)
    delta = float(np.abs(C1.astype(np.float64) - c0.astype(np.float64)).max())
    theta = 6.0 * delta + 1e-6
    if theta > THETA0:
        return None  # tracked margin radius insufficient; retry

    sel = mm_flat < np.float32(T)                  # [8, N_LOC] bool
    n_sel = int(sel.sum())
    if n_sel > MARGIN_CAP:
        return None

    # gather prescaled x rows for selected neurons
    core_idx, loc_idx = np.nonzero(sel)
    p_idx = loc_idx // FD
    j_idx = loc_idx % FD
    b_idx = loc_idx // F_LOC
    f_idx = core_idx * F_LOC + (loc_idx % F_LOC)
    x_rows = np.ascontiguousarray(xp[b_idx, f_idx, :], dtype=np.float32)

    # device-replica sim under c0: must reproduce the device bitwise
    sim0 = _sim_neurons(x_rows, chat0)

    spk_cores = [r[k]["spk"] for k in range(N_CORES)]
    tbv, gv = np.divmod(np.arange(T), G)
    dev_rows = np.zeros((n_sel, T), np.float32)
    for i in range(n_sel):
        sk = spk_cores[core_idx[i]]
        dev_rows[i] = sk[tbv, p_idx[i], gv * FD + j_idx[i]]
    mismatch = int((dev_rows != sim0["spikes"]).sum())
    if mismatch:
        sys.stderr.write(
            f"kernel.py: host/device replica mismatch ({mismatch} elems), "
            "falling back\n")
        return None

    # iterate the tiny fixed point on the selected set (scaled sums)
    base_spk = s_spk - sim0["spikes"].astype(np.float64).sum(axis=0)
    base_u = s_u - sim0["u"].astype(np.float64).sum(axis=0)
    base_w = s_w - sim0["w"].astype(np.float64).sum(axis=0)
    xtra0 = float(sum(rk["xtra"][0, 0] for rk in r))
    base_mlast = xtra0 - float(sim0["m_last"].astype(np.float64).sum())

    C_cur = C1
    sim = sim0
    converged = False
    for _ in range(12):
        chat_cur = (C_cur.astype(np.float64) / A64[:T]).astype(np.float32)
        sim = _sim_neurons(x_rows, chat_cur)
        s_spk_new = base_spk + sim["spikes"].astype(np.float64).sum(axis=0)
        C_next, _ = _ema_chain_from_sums(s_spk_new, base_v, bias_v)
        if np.array_equal(C_next.view(np.uint32), C_cur.view(np.uint32)):
            converged = True
            break
        C_cur = C_next
    if not converged:
        return None

    # safety: all excursions must stay inside the tracked margin radius
    exc = float(np.abs(C_cur.astype(np.float64) -
                       c0.astype(np.float64)).max())
    if exc > theta / 2.0:
        return None

    s_spk = base_spk + sim["spikes"].astype(np.float64).sum(axis=0)
    s_u = base_u + sim["u"].astype(np.float64).sum(axis=0)
    s_w = base_w + sim["w"].astype(np.float64).sum(axis=0)
    mlast_s = base_mlast + float(sim["m_last"].astype(np.float64).sum())
    _, ema_final = _ema_chain_from_sums(s_spk, base_v, bias_v)

    spikes = _unshard_spikes(spk_cores)
    spikes[b_idx, f_idx, :] = sim["spikes"]

    s_mem, s_g, mlast = _unscale_sums(s_u, s_w, mlast_s)
    return _assemble(sxp, spikes, s_mem, s_g, mlast, C_cur, ema_final)


def _assemble(sxp, spikes, s_mem, s_g, mem_last_sum, C_used, ema_final):
    # syn sums reconstructed from prescaled-input sums (f64 model):
    # S_t = S_{t-1} + sum(x'_t);  syn_t = beta^t * S_t
    s_syn = np.zeros(T, np.float64)
    run = 0.0
    for t in range(T):
        run = run + sxp[t]
        s_syn[t] = B64[t] * run

    mem_post = np.empty(T, np.float64)
    mem_post[:T - 1] = (s_mem[1:] - s_syn[1:]) / np.float64(ALPHA)
    mem_post[T - 1] = mem_last_sum
    mem_trace = (mem_post / N_GLOB).astype(np.float32)

    thr_trace = (C_used.astype(np.float64)
                 + s_g / (np.float64(GAMMA) * N_GLOB)).astype(np.float32)

    adaptation_mean = np.float32(
        s_g[T - 1] / (0.1 * np.float64(GAMMA)) / N_GLOB)

    return (spikes, mem_trace, thr_trace, np.float32(ema_final),
            adaptation_mean)


def _kernel_multipass(xs, sxp, c0, base_v, bias_v):
    crows, _ = _chat_rows(c0)
    r = _run_device(xs, crows, base_v, bias_v, 2)
    acc = np.stack([rk["acc"] for rk in r])
    g_sums = acc.astype(np.float64).sum(axis=(0, 2))
    s_spk, s_u, s_w = g_sums
    c_dev = r[0]["crow"][1].astype(np.float32)
    _, ema_final = _ema_chain_from_sums(s_spk, base_v, bias_v)
    xtra_sum = float(sum(rk["xtra"][0, 0] for rk in r))
    spikes = _unshard_spikes([rk["spk"] for rk in r])
    s_mem, s_g, mlast = _unscale_sums(s_u, s_w, xtra_sum)
    return _assemble(sxp, spikes, s_mem, s_g, mlast, c_dev, ema_final)


if __name__ == "__main__":
    rng = np.random.default_rng(0)
    x = rng.standard_normal((B, F, T)).astype(np.float32)
    t0 = time.time()
    out = kernel(x, np.ones(1, np.float32), np.zeros(1, np.float32))
    print("kernel wall:", time.time() - t0)
    for o in out:
        print(np.shape(o), np.asarray(o).dtype)


# revision 24
# speedup vs baseline: 1.1920x; 1.1920x over previous
"""Adaptive-threshold LIF neuron recurrence (SNN) on 8 Trainium2 NeuronCores.

Strategy
--------
The recurrence is data-parallel over the 262144 neurons except for a scalar
firing-rate EMA that couples every neuron at every timestep (the spike MEAN
feeds the next step's threshold).  A per-step AllReduce would cost ~10us x
128 steps, so instead:

  host:   estimate the threshold-offset sequence C0[t] from a neuron
          subsample (cheap numpy sim)
  device: ONE data-parallel pass with C0 per core, recording per-step
          spike/membrane/adaptation partial sums (exact f32 integers for
          spikes) AND a per-neuron count of "near-margin" steps
          (|u - C0| <= theta0)
  host:   compute the exact global EMA chain from the (integer) spike sums
          -> corrected C[t]; every neuron with no near-margin step provably
          has an identical trajectory under the corrected C; the few
          remaining neurons are re-simulated in numpy with bit-exact
          replicas of the device arithmetic, iterating the tiny fixed point
          until the C chain is stable; spikes and sums are patched.

To minimize per-step work on the (bottleneck) Vector engine, the state is
kept in geometrically rescaled coordinates with compile-time per-step
scale immediates:

  x'_t   = x_t / beta^t                (host prescale)
  S_t    = S_{t-1} + x'_t              (GPSIMD add; == syn_t/beta^t)
  synv_t = (beta^t/alpha^t) * S_t      (ACT copy; == syn_t/alpha^t)
  m_t    = m'_{t-1} + synv_t           (GPSIMD add; m == mem/alpha^t)
  u_t    = k1*m_t - W_{t-1}            (V stt, accum; u == (mem-.1*adapt)/a^t)
  spike  = u_t >= C'_t                 (V tensor_scalar, accum; C' = C/a^t)
  e_t    = relu(u_t - C'_t)            (ACT relu, bias=-C')
  m'_t   = spike ? e_t : m_t           (V copy_predicated)
  tmp    = (kk/alpha)*u_t              (ACT copy)
  W_t    = (g2/alpha)*W_{t-1} + tmp    (V stt, accum; W == 0.1*gamma*adapt
                                        / alpha^{t+1})
  ab     = |u_t - C'_t|                (ACT abs)
  sgn    = sign(ab - theta0/alpha^t)   (ACT sign; -1/0 when near margin)
  asum  += sgn                         (GPSIMD add; == T iff provably safe)

Spike sums are sums of 0/1 values, so every f32 reduction of them is exact
(integers < 2^24), which makes the host EMA chain exact.  A 2-pass fully
on-device variant (pass1 -> AllReduce -> pass2) is kept as a fallback.

Per-core layout: features sharded 8 ways (1024 per core), time-major tiles
x[tb, p, g*256+j] = x'[t = tb*G+g, neuron n = p*256+j], n = b*1024+f.
"""

import math
import sys
import time

if "/opt/trn_rl_repo" not in sys.path:
    sys.path.insert(0, "/opt/trn_rl_repo")

import numpy as np

# ---------------------------------------------------------------- constants
B, F, T = 32, 8192, 128
N_CORES = 8
F_LOC = F // N_CORES            # 1024 features per core
N_LOC = B * F_LOC               # 32768 neurons per core
N_GLOB = B * F                  # 262144 neurons total
P = 128                         # SBUF partitions
FD = N_LOC // P                 # 256 free-dim elems per step tile
G = 16                          # timesteps per DMA group
TB = T // G                     # 8 groups

# exact f32 constants as produced by the jax reference (verified bitwise)
ALPHA = np.uint32(0x3F7383C5).view(np.float32)  # exp(-.001/.02)
BETA = np.uint32(0x3F519857).view(np.float32)   # exp(-.001/.005)
GAMMA = np.uint32(0x3F7D73E8).view(np.float32)  # exp(-.001/.1)
C1G = np.uint32(0x3C230600).view(np.float32)    # 1 - gamma
ADAPT_STRENGTH = np.float32(0.1)
HOMEO_RATE = np.float32(0.01)
THRESHOLD_BASE = np.float32(1.0)

K1 = np.float32(1.0 - 0.1 * float(C1G))            # 1 - 0.1*(1-gamma)
K2 = np.float32(0.1 * float(GAMMA) * float(C1G))   # 0.1*gamma*(1-gamma)
# adaptation state W ~ 0.1*gamma*adapt, updated from u:
#   G_t = g2*G_{t-1} + kk*u_t
KK = np.float32(float(K2) / float(K1))
G2 = np.float32(float(GAMMA) + float(K2) / float(K1))

# per-step geometric scales
A64 = np.float64(ALPHA) ** np.arange(T + 1)      # alpha^t
B64 = np.float64(BETA) ** np.arange(T)           # beta^t
IB64 = 1.0 / B64                                 # input prescale
SV32 = (B64 / A64[:T]).astype(np.float32)        # synv scale immediates
KKA = np.float32(float(KK) / float(ALPHA))
G2A = np.float32(float(G2) / float(ALPHA))

N_PASSES = 1          # 1 = single pass + host margin correction (default)
MARGIN_CAP = 40000    # fall back to a device re-run above this many neurons
THETA0 = 3e-4         # margin radius tracked on device (unscaled u units)

_BUILD_CACHE = {}


# ------------------------------------------------------------- device build
def _build_kernel(n_passes=N_PASSES, base=1.0, bias=0.0):
    key = (n_passes, float(base), float(bias))
    if key in _BUILD_CACHE:
        return _BUILD_CACHE[key]

    import concourse.bacc as bacc
    import concourse.mybir as mybir
    from concourse import tile

    DT = mybir.dt.float32
    AF = mybir.ActivationFunctionType
    OP = mybir.AluOpType
    I32 = mybir.dt.int32

    nc = bacc.Bacc(None, target_bir_lowering=False, debug=False,
                   num_devices=N_CORES)

    x_in = nc.dram_tensor("x", [TB, P, G * FD], DT, kind="ExternalInput")
    # rows: 0 = C' (scaled C), 1 = -C', 2 = -theta0/alpha^t, 3 = 1/alpha^t
    c0_in = nc.dram_tensor("c0", [4, T], DT, kind="ExternalInput")

    spk_out = nc.dram_tensor("spk", [TB, P, G * FD], DT, kind="ExternalOutput")
    acc_out = nc.dram_tensor("acc", [3, P, T], DT, kind="ExternalOutput")
    crow_out = nc.dram_tensor("crow", [max(n_passes, 2), T], DT,
                              kind="ExternalOutput")
    xtra_out = nc.dram_tensor("xtra", [1, 8], DT, kind="ExternalOutput")
    mm_out = nc.dram_tensor("mm", [P, FD], DT, kind="ExternalOutput")

    with tile.TileContext(nc) as tc:
        with (
            tc.tile_pool(name="state", bufs=1) as st,
            tc.tile_pool(name="xload", bufs=3) as xl,
            tc.tile_pool(name="sout", bufs=2) as so,
            tc.tile_pool(name="psum", bufs=2, space="PSUM") as ps,
            tc.tile_pool(name="dram", bufs=1, space="DRAM") as dram,
        ):
            mem = st.tile([P, FD], DT, tag="mem")
            gst = st.tile([P, FD], DT, tag="gst")
            mm = st.tile([P, FD], DT, tag="mm")
            spk_s = st.tile([P, FD], DT, tag="spk_s")
            ones = st.tile([P, 1], DT, tag="ones")
            nc.vector.memset(ones[:], 1.0)
            track_margin = n_passes == 1
            if track_margin:
                nc.gpsimd.memset(mm[:], 0.0)

            # broadcast C rows ([128, T]): positive scaled C for the spike
            # compare, negated scaled C for ACT bias, scaled margin radius
            cmat = [
                st.tile([P, T], DT, tag=f"cmat{p}", name=f"cmat{p}")
                for p in range(n_passes)
            ]
            nmat = [
                st.tile([P, T], DT, tag=f"nmat{p}", name=f"nmat{p}")
                for p in range(n_passes)
            ]
            thmat = st.tile([P, T], DT, tag="thmat")
            c0row = st.tile([1, T], DT, tag="c0row")
            ncrow = st.tile([1, T], DT, tag="ncrow")
            throw = st.tile([1, T], DT, tag="throw")
            iarow = st.tile([1, T], DT, tag="iarow")
            nc.sync.dma_start(c0row[:], c0_in[0:1, :])
            nc.sync.dma_start(ncrow[:], c0_in[1:2, :])
            nc.sync.dma_start(throw[:], c0_in[2:3, :])
            nc.sync.dma_start(iarow[:], c0_in[3:4, :])
            nc.gpsimd.partition_broadcast(cmat[0][:], c0row[:], channels=P)
            nc.gpsimd.partition_broadcast(nmat[0][:], ncrow[:], channels=P)
            nc.gpsimd.partition_broadcast(thmat[:], throw[:], channels=P)

            # accumulators: [128, T]: spike counts, u sums, W sums
            acc_spk = st.tile([P, T], DT, tag="acc_spk")
            acc_u = st.tile([P, T], DT, tag="acc_u")
            acc_w = st.tile([P, T], DT, tag="acc_w")

            # constant row of 0.99 for the ema scan (2-pass mode)
            row99 = st.tile([1, T], DT, tag="row99")
            nc.vector.memset(row99[:], 0.99)

            for p_i in range(n_passes):
                last = p_i == n_passes - 1
                C = cmat[p_i]
                NC = nmat[p_i]
                syn = st.tile([P, FD], DT, tag="syn", name="syn0", bufs=2)
                nc.gpsimd.memset(syn[:], 0.0)
                nc.gpsimd.memset(mem[:], 0.0)
                nc.vector.memset(gst[:], 0.0)

                for tb in range(TB):
                    if tb == 0:
                        # split the first load 4+2 ways so compute starts
                        # as soon as the first column chunk lands
                        xt_a = xl.tile([P, 4 * FD], DT, tag="xta",
                                       name="xta")
                        for q in range(4):
                            nc.sync.dma_start(
                                xt_a[:, q * FD:(q + 1) * FD],
                                x_in[0, :, q * FD:(q + 1) * FD])
                        xt_b = xl.tile([P, 12 * FD], DT, tag="xtb",
                                       name="xtb")
                        nc.sync.dma_start(xt_b[:, 0:6 * FD],
                                          x_in[0, :, 4 * FD:10 * FD])
                        nc.sync.dma_start(xt_b[:, 6 * FD:],
                                          x_in[0, :, 10 * FD:])
                    else:
                        xt = xl.tile([P, G * FD], DT, tag="xt", name="xt")
                        nc.sync.dma_start(xt[:], x_in[tb, :, :])
                    if last:
                        ot = so.tile([P, G * FD], DT, tag="ot")
                    for g in range(G):
                        t = tb * G + g
                        if tb == 0:
                            if g < 4:
                                xs = xt_a[:, g * FD:(g + 1) * FD]
                            else:
                                xs = xt_b[:, (g - 4) * FD:(g - 3) * FD]
                        else:
                            xs = xt[:, g * FD:(g + 1) * FD]
                        u = st.tile([P, FD], DT, tag="u", name="u", bufs=2)
                        e = st.tile([P, FD], DT, tag="e", name="e", bufs=2)
                        tmp = st.tile([P, FD], DT, tag="tmp", name="tmp",
                                      bufs=2)
                        ab = st.tile([P, FD], DT, tag="ab", name="ab",
                                     bufs=2)
                        sg = st.tile([P, FD], DT, tag="sg", name="sg",
                                     bufs=2)
                        # S_t = S_{t-1} + x'_t (GPSIMD, double-buffered)
                        syn_new = st.tile([P, FD], DT, tag="syn",
                                          name="syn", bufs=2)
                        nc.gpsimd.tensor_tensor(syn_new[:], syn[:], xs,
                                                OP.add)
                        syn = syn_new
                        # synv_t = (beta^t/alpha^t) * S_t (ACT)
                        synu = st.tile([P, FD], DT, tag="synu", name="synu",
                                       bufs=2)
                        nc.scalar.activation(synu[:], syn[:], AF.Copy,
                                             scale=float(SV32[t]))
                        # m_t = m'_{t-1} + synv_t (V tensor_tensor: keeps
                        # the reset->membrane->u loop on one engine)
                        nc.vector.tensor_tensor(mem[:], mem[:], synu[:],
                                                OP.add)
                        # u_t = k1*m - W (V stt, accum -> u sums)
                        nc.vector.scalar_tensor_tensor(
                            u[:], mem[:], float(K1), gst[:],
                            OP.mult, OP.subtract,
                            accum_out=acc_u[:, t:t + 1] if last else None,
                        )
                        # reset value e = relu(u + (-C')) (ACT; on the
                        # critical V loop so emitted early)
                        nc.scalar.activation(
                            e[:], u[:], AF.Relu, bias=NC[:, t:t + 1],
                            scale=1.0,
                        )
                        if last:
                            sp_ap = ot[:, g * FD:(g + 1) * FD]
                        else:
                            sp_ap = spk_s[:]
                        # spike = (u >= C'), accum(reduce add) -> step sums
                        nc.vector.tensor_scalar(
                            sp_ap, u[:], C[:, t:t + 1], 0.0,
                            OP.is_ge, OP.add,
                            accum_out=acc_spk[:, t:t + 1],
                        )
                        nc.vector.copy_predicated(
                            mem[:], sp_ap.bitcast(I32), e[:]
                        )
                        # W_t = (g2/a)*W + (kk/a)*u (ACT + V stt, accum)
                        nc.scalar.activation(tmp[:], u[:], AF.Copy,
                                             scale=float(KKA))
                        nc.vector.scalar_tensor_tensor(
                            gst[:], gst[:], float(G2A), tmp[:],
                            OP.mult, OP.add,
                            accum_out=acc_w[:, t:t + 1] if last else None,
                        )
                        if track_margin:
                            nc.scalar.activation(
                                ab[:], u[:], AF.Abs, bias=NC[:, t:t + 1],
                                scale=1.0,
                            )
                            nc.scalar.activation(
                                sg[:], ab[:], AF.Sign,
                                bias=thmat[:, t:t + 1], scale=1.0,
                            )
                            nc.gpsimd.tensor_tensor(mm[:], mm[:], sg[:],
                                                    OP.add)
                    if last:
                        nc.sync.dma_start(spk_out[tb, :, :], ot[:])

                # ---- inter-pass: global spike sums -> next C (2-pass mode)
                if not last:
                    pt = ps.tile([1, T], DT, tag="pt")
                    nc.tensor.matmul(pt[:], ones[:], acc_spk[:],
                                     start=True, stop=True)
                    srow = st.tile([1, T], DT, tag=f"srow{p_i}",
                                   name=f"srow{p_i}")
                    nc.scalar.copy(srow[:], pt[:])
                    cc_in = dram.tile([1, T], DT, name=f"ccin{p_i}")
                    cc_out = dram.tile([1, T], DT, name=f"ccout{p_i}")
                    nc.sync.dma_start(cc_in[:], srow[:])
                    nc.gpsimd.collective_compute(
                        "AllReduce", OP.add,
                        replica_groups=[list(range(N_CORES))],
                        ins=[cc_in.opt()], outs=[cc_out.opt()],
                    )
                    sglob = st.tile([1, T], DT, tag=f"sglob{p_i}",
                                    name=f"sglob{p_i}")
                    nc.sync.dma_start(sglob[:], cc_out[:])
                    # ema chain: msc = sums * (0.01/N_GLOB); scan; homeo; C
                    mean_scale = np.float32(
                        np.float32(0.01) * np.float32(1.0 / N_GLOB))
                    msc = st.tile([1, T], DT, tag=f"msc{p_i}",
                                  name=f"msc{p_i}")
                    nc.vector.tensor_scalar(
                        msc[:], sglob[:], float(mean_scale), None, OP.mult)
                    ema = st.tile([1, T], DT, tag=f"ema{p_i}",
                                  name=f"ema{p_i}")
                    nc.vector.tensor_tensor_scan(
                        ema[:], row99[:], msc[:], 0.0, OP.mult, OP.add)
                    hm = st.tile([1, T], DT, tag=f"hm{p_i}", name=f"hm{p_i}")
                    nc.vector.tensor_scalar(
                        hm[:], ema[:], float(HOMEO_RATE), 0.0,
                        OP.subtract, OP.max)
                    cn = st.tile([1, T], DT, tag=f"cn{p_i}", name=f"cn{p_i}")
                    # C[0] is pass-independent (ema starts at 0; scale=1)
                    nc.vector.tensor_copy(cn[:, 0:1], c0row[:, 0:1])
                    nc.vector.tensor_scalar(
                        cn[:, 1:T], hm[:, 0:T - 1], float(ADAPT_STRENGTH),
                        float(base), OP.mult, OP.add)
                    if float(bias) != 0.0:
                        nc.vector.tensor_scalar(
                            cn[:, 1:T], cn[:, 1:T], float(bias), None,
                            OP.add)
                    # scale to C' = C/alpha^t (input row 3 = 1/alpha^t)
                    csc = st.tile([1, T], DT, tag=f"csc{p_i}",
                                  name=f"csc{p_i}")
                    nc.vector.tensor_tensor(csc[:], cn[:], iarow[:],
                                            OP.mult)
                    ncn = st.tile([1, T], DT, tag=f"ncn{p_i}",
                                  name=f"ncn{p_i}")
                    nc.vector.tensor_scalar(
                        ncn[:], csc[:], -1.0, None, OP.mult)
                    nc.gpsimd.partition_broadcast(
                        cmat[p_i + 1][:], csc[:], channels=P)
                    nc.gpsimd.partition_broadcast(
                        nmat[p_i + 1][:], ncn[:], channels=P)
                    nc.sync.dma_start(crow_out[p_i + 1:p_i + 2, :], cn[:])

            # ---- final extras: sum of post-reset scaled mem at t=T-1
            memsum = st.tile([P, 1], DT, tag="memsum")
            nc.vector.reduce_sum(memsum[:], mem[:], axis=mybir.AxisListType.X)
            ptx = ps.tile([1, 1], DT, tag="ptx")
            nc.tensor.matmul(ptx[:], ones[:], memsum[:], start=True, stop=True)
            xrow = st.tile([1, 8], DT, tag="xrow")
            nc.vector.memset(xrow[:], 0.0)
            nc.scalar.copy(xrow[:, 0:1], ptx[:])
            nc.sync.dma_start(xtra_out[:, :], xrow[:])

            # C' used in pass 0 (echo back for debugging)
            nc.sync.dma_start(crow_out[0:1, :], c0row[:, :])

            # accumulators + margin counters out
            nc.sync.dma_start(acc_out[0, :, :], acc_spk[:])
            nc.sync.dma_start(acc_out[1, :, :], acc_u[:])
            nc.sync.dma_start(acc_out[2, :, :], acc_w[:])
            if track_margin:
                nc.sync.dma_start(mm_out[:, :], mm[:])
            else:
                nc.sync.dma_start(mm_out[:, :], spk_s[:])

    nc.compile()
    _BUILD_CACHE[key] = nc
    return nc


# --------------------------------------------------------------- host logic
def _ema_chain_from_sums(sums, base, bias):
    """Replicate the reference's scalar f32 EMA/homeo chain from global
    per-step spike sums (exact integers).  Returns (C[t] used AT step t,
    final ema)."""
    ema = np.float32(0.0)
    C = np.zeros(T, np.float32)
    for t in range(T):
        homeo = np.float32(ADAPT_STRENGTH * max(np.float32(ema - HOMEO_RATE),
                                                np.float32(0.0)))
        C[t] = np.float32(np.float32(base + homeo) + bias)
        mean = np.float32(np.float32(sums[t]) * np.float32(1.0 / N_GLOB))
        ema = np.float32(math.fma(float(np.float32(0.99)), float(ema),
                                  float(np.float32(np.float32(0.01) * mean))))
    return C, ema


def _estimate_c0(input_current, base, bias, stride=16):
    """Cheap numpy f32 simulation of a strided neuron subsample with a local
    EMA to estimate the C[t] threshold-offset sequence."""
    x = np.ascontiguousarray(
        input_current[:, ::stride, :], dtype=np.float32)  # [B, F/stride, T]
    nb, nf, nt = x.shape
    x = x.reshape(nb * nf, nt)
    syn = np.zeros(nb * nf, np.float32)
    mem = np.zeros(nb * nf, np.float32)
    adp = np.zeros(nb * nf, np.float32)
    ema = np.float32(0.0)
    C = np.zeros(nt, np.float32)
    for t in range(nt):
        syn = BETA * syn + x[:, t]
        mem = ALPHA * mem + syn
        homeo = np.float32(ADAPT_STRENGTH * max(np.float32(ema - HOMEO_RATE),
                                                np.float32(0.0)))
        adp = GAMMA * adp + C1G * mem
        C[t] = np.float32(np.float32(base + homeo) + bias)
        thr = C[t] + ADAPT_STRENGTH * adp
        spike = (mem >= thr)
        ema = np.float32(0.99 * ema + 0.01 * np.float32(spike.mean()))
        mem = np.where(spike, mem - thr, mem).astype(np.float32)
    return C


def _chat_rows(C):
    """Device input rows for a given (unscaled) C chain: [C', -C',
    -theta0/alpha^t, 1/alpha^t], all f32 [4, T]."""
    chat = (C.astype(np.float64) / A64[:T]).astype(np.float32)
    th = (np.float64(THETA0) / A64[:T]).astype(np.float32)
    ia = (1.0 / A64[:T]).astype(np.float32)
    return np.stack([chat, -chat, -th, ia]).astype(np.float32), chat


def _sim_neurons(x_rows, chat):
    """Bit-exact numpy replica of the device per-neuron chain (scaled
    coordinates).  x_rows: [n, T] f32 prescaled inputs.  chat: [T] f32
    scaled C' sequence.  Returns per-step spikes/u/W plus final m'."""
    n = x_rows.shape[0]
    syn = np.zeros(n, np.float32)
    mem = np.zeros(n, np.float32)
    gst = np.zeros(n, np.float32)
    spikes = np.zeros((n, T), np.float32)
    u_tr = np.zeros((n, T), np.float32)
    w_tr = np.zeros((n, T), np.float32)
    for t in range(T):
        syn = (syn + x_rows[:, t]).astype(np.float32)
        synv = (syn * SV32[t]).astype(np.float32)
        mem = (mem + synv).astype(np.float32)
        u = ((mem * K1).astype(np.float32) - gst).astype(np.float32)
        u_tr[:, t] = u
        sp = u >= chat[t]
        spikes[:, t] = sp
        ev = np.maximum((u - chat[t]).astype(np.float32), np.float32(0.0))
        mem = np.where(sp, ev, mem).astype(np.float32)
        tmp = (u * KKA).astype(np.float32)
        gst = ((gst * G2A).astype(np.float32) + tmp).astype(np.float32)
        w_tr[:, t] = gst
    return {"spikes": spikes, "u": u_tr, "w": w_tr, "m_last": mem}


def _prescale(input_current):
    """x'_[b,f,t] = f32(x / beta^t) plus its per-step global f64 sums."""
    x64 = input_current.astype(np.float64) * IB64[None, None, :]
    xp = x64.astype(np.float32)
    sxp = x64.sum(axis=(0, 1))
    return xp, sxp


def _shard_input(xp):
    """prescaled [B, F, T] -> per-core [TB, P, G*FD] time-major tiles."""
    xs = []
    for k in range(N_CORES):
        xk = xp[:, k * F_LOC:(k + 1) * F_LOC, :]  # [B, F_LOC, T]
        xk = np.ascontiguousarray(np.moveaxis(xk, -1, 0)).reshape(T, N_LOC)
        xk = xk.reshape(TB, G, P, FD).transpose(0, 2, 1, 3)
        xs.append(np.ascontiguousarray(xk.reshape(TB, P, G * FD),
                                       dtype=np.float32))
    return xs


def _unshard_spikes(spk_cores):
    """per-core [TB, P, G*FD] -> [B, F, T] float32."""
    out = np.empty((B, F, T), np.float32)
    for k, sk in enumerate(spk_cores):
        sk = sk.reshape(TB, P, G, FD).transpose(0, 2, 1, 3).reshape(T, N_LOC)
        sk = np.moveaxis(sk.reshape(T, B, F_LOC), 0, -1)
        out[:, k * F_LOC:(k + 1) * F_LOC, :] = sk
    return out


def _run_device(xs, crows, base, bias, n_passes):
    from concourse.bass_utils import run_bass_kernel_spmd

    nc = _build_kernel(n_passes, float(base), float(bias))
    in_maps = [{"x": xs[k], "c0": crows} for k in range(N_CORES)]
    res = run_bass_kernel_spmd(nc, in_maps, core_ids=list(range(N_CORES)))
    return res.results


def _unscale_sums(s_u, s_w, xtra_sum):
    """Scaled per-step u/W sums -> unscaled mem_pre / G sums (f64)."""
    s_w_prev = np.concatenate([[0.0], s_w[:-1]])
    s_mem = A64[:T] * (s_u + s_w_prev) / np.float64(K1)
    s_g = A64[1:T + 1] * s_w
    mlast = xtra_sum * A64[T - 1]
    return s_mem, s_g, mlast


def kernel(input_current, threshold_scale, adaptation_bias):
    input_current = np.asarray(input_current, np.float32)
    threshold_scale = np.asarray(threshold_scale, np.float32)
    adaptation_bias = np.asarray(adaptation_bias, np.float32)
    assert input_current.shape == (B, F, T)

    scale_v = np.float32(threshold_scale.reshape(-1)[0])
    bias_v = np.float32(adaptation_bias.reshape(-1)[0])
    base_v = np.float32(THRESHOLD_BASE * scale_v)

    c0 = _estimate_c0(input_current, base_v, bias_v)
    xp, sxp = _prescale(input_current)
    xs = _shard_input(xp)

    if N_PASSES != 1:
        return _kernel_multipass(xs, sxp, c0, base_v, bias_v)

    for attempt in range(3):
        crows, chat0 = _chat_rows(c0)
        r = _run_device(xs, crows, base_v, bias_v, 1)
        out = _finish_single(xp, sxp, c0, chat0, base_v, bias_v, r)
        if out is not None:
            return out
        # margin set too large -> c0 was far off; retry with corrected C
        acc = np.stack([rk["acc"] for rk in r])
        s_spk = acc.astype(np.float64).sum(axis=(0, 2))[0]
        c0, _ = _ema_chain_from_sums(s_spk, base_v, bias_v)
    # last resort: fully on-device 2-pass mode
    return _kernel_multipass(xs, sxp, c0, base_v, bias_v)


def _finish_single(xp, sxp, c0, chat0, base_v, bias_v, r):
    acc = np.stack([rk["acc"] for rk in r])       # [8, 3, P, T]
    g_sums = acc.astype(np.float64).sum(axis=(0, 2))
    s_spk, s_u, s_w = g_sums
    mm = np.stack([rk["mm"] for rk in r])          # [8, P, FD]
    mm_flat = mm.reshape(N_CORES, N_LOC)           # neuron n = p*FD + j

    # candidate corrected C chain
    C1, _ = _ema_chain_from_sums(s_spk, base_v, bias_v)
    delta = float(np.abs(C1.astype(np.float64) - c0.astype(np.float64)).max())
    theta = 6.0 * delta + 1e-6
    if theta > THETA0:
        return None  # tracked margin radius insufficient; retry

    sel = mm_flat < np.float32(T)                  # [8, N_LOC] bool
    n_sel = int(sel.sum())
    if n_sel > MARGIN_CAP:
        return None

    # gather prescaled x rows for selected neurons
    core_idx, loc_idx = np.nonzero(sel)
    p_idx = loc_idx // FD
    j_idx = loc_idx % FD
    b_idx = loc_idx // F_LOC
    f_idx = core_idx * F_LOC + (loc_idx % F_LOC)
    x_rows = np.ascontiguousarray(xp[b_idx, f_idx, :], dtype=np.float32)

    # device-replica sim under c0: must reproduce the device bitwise
    sim0 = _sim_neurons(x_rows, chat0)

    spk_cores = [r[k]["spk"] for k in range(N_CORES)]
    tbv, gv = np.divmod(np.arange(T), G)
    dev_rows = np.zeros((n_sel, T), np.float32)
    for i in range(n_sel):
        sk = spk_cores[core_idx[i]]
        dev_rows[i] = sk[tbv, p_idx[i], gv * FD + j_idx[i]]
    mismatch = int((dev_rows != sim0["spikes"]).sum())
    if mismatch:
        sys.stderr.write(
            f"kernel.py: host/device replica mismatch ({mismatch} elems), "
            "falling back\n")
        return None

    # iterate the tiny fixed point on the selected set (scaled sums)
    base_spk = s_spk - sim0["spikes"].astype(np.float64).sum(axis=0)
    base_u = s_u - sim0["u"].astype(np.float64).sum(axis=0)
    base_w = s_w - sim0["w"].astype(np.float64).sum(axis=0)
    xtra0 = float(sum(rk["xtra"][0, 0] for rk in r))
    base_mlast = xtra0 - float(sim0["m_last"].astype(np.float64).sum())

    C_cur = C1
    sim = sim0
    converged = False
    for _ in range(12):
        chat_cur = (C_cur.astype(np.float64) / A64[:T]).astype(np.float32)
        sim = _sim_neurons(x_rows, chat_cur)
        s_spk_new = base_spk + sim["spikes"].astype(np.float64).sum(axis=0)
        C_next, _ = _ema_chain_from_sums(s_spk_new, base_v, bias_v)
        if np.array_equal(C_next.view(np.uint32), C_cur.view(np.uint32)):
            converged = True
            break
        C_cur = C_next
    if not converged:
        return None

    # safety: all excursions must stay inside the tracked margin radius
    exc = float(np.abs(C_cur.astype(np.float64) -
                       c0.astype(np.float64)).max())
    if exc > theta / 2.0:
        return None

    s_spk = base_spk + sim["spikes"].astype(np.float64).sum(axis=0)
    s_u = base_u + sim["u"].astype(np.float64).sum(axis=0)
    s_w = base_w + sim["w"].astype(np.float64).sum(axis=0)
    mlast_s = base_mlast + float(sim["m_last"].astype(np.float64).sum())
    _, ema_final = _ema_chain_from_sums(s_spk, base_v, bias_v)

    spikes = _unshard_spikes(spk_cores)
    spikes[b_idx, f_idx, :] = sim["spikes"]

    s_mem, s_g, mlast = _unscale_sums(s_u, s_w, mlast_s)
    return _assemble(sxp, spikes, s_mem, s_g, mlast, C_cur, ema_final)


def _assemble(sxp, spikes, s_mem, s_g, mem_last_sum, C_used, ema_final):
    # syn sums reconstructed from prescaled-input sums (f64 model):
    # S_t = S_{t-1} + sum(x'_t);  syn_t = beta^t * S_t
    s_syn = np.zeros(T, np.float64)
    run = 0.0
    for t in range(T):
        run = run + sxp[t]
        s_syn[t] = B64[t] * run

    mem_post = np.empty(T, np.float64)
    mem_post[:T - 1] = (s_mem[1:] - s_syn[1:]) / np.float64(ALPHA)
    mem_post[T - 1] = mem_last_sum
    mem_trace = (mem_post / N_GLOB).astype(np.float32)

    thr_trace = (C_used.astype(np.float64)
                 + s_g / (np.float64(GAMMA) * N_GLOB)).astype(np.float32)

    adaptation_mean = np.float32(
        s_g[T - 1] / (0.1 * np.float64(GAMMA)) / N_GLOB)

    return (spikes, mem_trace, thr_trace, np.float32(ema_final),
            adaptation_mean)


def _kernel_multipass(xs, sxp, c0, base_v, bias_v):
    crows, _ = _chat_rows(c0)
    r = _run_device(xs, crows, base_v, bias_v, 2)
    acc = np.stack([rk["acc"] for rk in r])
    g_sums = acc.astype(np.float64).sum(axis=(0, 2))
    s_spk, s_u, s_w = g_sums
    c_dev = r[0]["crow"][1].astype(np.float32)
    _, ema_final = _ema_chain_from_sums(s_spk, base_v, bias_v)
    xtra_sum = float(sum(rk["xtra"][0, 0] for rk in r))
    spikes = _unshard_spikes([rk["spk"] for rk in r])
    s_mem, s_g, mlast = _unscale_sums(s_u, s_w, xtra_sum)
    return _assemble(sxp, spikes, s_mem, s_g, mlast, c_dev, ema_final)


if __name__ == "__main__":
    rng = np.random.default_rng(0)
    x = rng.standard_normal((B, F, T)).astype(np.float32)
    t0 = time.time()
    out = kernel(x, np.ones(1, np.float32), np.zeros(1, np.float32))
    print("kernel wall:", time.time() - t0)
    for o in out:
        print(np.shape(o), np.asarray(o).dtype)


# revision 27
# speedup vs baseline: 1.2290x; 1.0310x over previous
"""Adaptive-threshold LIF neuron recurrence (SNN) on 8 Trainium2 NeuronCores.

Strategy
--------
The recurrence is data-parallel over the 262144 neurons except for a scalar
firing-rate EMA that couples every neuron at every timestep (the spike MEAN
feeds the next step's threshold).  A per-step AllReduce would cost ~10us x
128 steps, so instead:

  host:   estimate the threshold-offset sequence C0[t] from a neuron
          subsample (cheap numpy sim)
  device: ONE data-parallel pass with C0 per core, recording per-step
          spike/membrane/adaptation partial sums (exact f32 integers for
          spikes) AND a per-neuron count of "near-margin" steps
          (|u - C0| <= theta0)
  host:   compute the exact global EMA chain from the (integer) spike sums
          -> corrected C[t]; every neuron with no near-margin step provably
          has an identical trajectory under the corrected C; the few
          remaining neurons are re-simulated in numpy with bit-exact
          replicas of the device arithmetic, iterating the tiny fixed point
          until the C chain is stable; spikes and sums are patched.

To minimize per-step work on the (bottleneck) Vector engine, the state is
kept in geometrically rescaled coordinates with compile-time per-step
scale immediates:

  x'_t   = x_t / beta^t                (host prescale)
  S_t    = S_{t-1} + x'_t              (GPSIMD add; == syn_t/beta^t)
  synv_t = (beta^t/alpha^t) * S_t      (ACT copy; == syn_t/alpha^t)
  m_t    = m'_{t-1} + synv_t           (GPSIMD add; m == mem/alpha^t)
  u_t    = k1*m_t - W_{t-1}            (V stt, accum; u == (mem-.1*adapt)/a^t)
  spike  = u_t >= C'_t                 (V tensor_scalar, accum; C' = C/a^t)
  e_t    = relu(u_t - C'_t)            (ACT relu, bias=-C')
  m'_t   = spike ? e_t : m_t           (V copy_predicated)
  tmp    = (kk/alpha)*u_t              (ACT copy)
  W_t    = (g2/alpha)*W_{t-1} + tmp    (V stt, accum; W == 0.1*gamma*adapt
                                        / alpha^{t+1})
  ab     = |u_t - C'_t|                (ACT abs)
  sgn    = sign(ab - theta0/alpha^t)   (ACT sign; -1/0 when near margin)
  asum  += sgn                         (GPSIMD add; == T iff provably safe)

Spike sums are sums of 0/1 values, so every f32 reduction of them is exact
(integers < 2^24), which makes the host EMA chain exact.  A 2-pass fully
on-device variant (pass1 -> AllReduce -> pass2) is kept as a fallback.

Per-core layout: features sharded 8 ways (1024 per core), time-major tiles
x[tb, p, g*256+j] = x'[t = tb*G+g, neuron n = p*256+j], n = b*1024+f.
"""

import math
import sys
import time

if "/opt/trn_rl_repo" not in sys.path:
    sys.path.insert(0, "/opt/trn_rl_repo")

import numpy as np

# ---------------------------------------------------------------- constants
B, F, T = 32, 8192, 128
N_CORES = 8
F_LOC = F // N_CORES            # 1024 features per core
N_LOC = B * F_LOC               # 32768 neurons per core
N_GLOB = B * F                  # 262144 neurons total
P = 128                         # SBUF partitions
FD = N_LOC // P                 # 256 free-dim elems per step tile
G = 16                          # timesteps per DMA group
TB = T // G                     # 8 groups

# exact f32 constants as produced by the jax reference (verified bitwise)
ALPHA = np.uint32(0x3F7383C5).view(np.float32)  # exp(-.001/.02)
BETA = np.uint32(0x3F519857).view(np.float32)   # exp(-.001/.005)
GAMMA = np.uint32(0x3F7D73E8).view(np.float32)  # exp(-.001/.1)
C1G = np.uint32(0x3C230600).view(np.float32)    # 1 - gamma
ADAPT_STRENGTH = np.float32(0.1)
HOMEO_RATE = np.float32(0.01)
THRESHOLD_BASE = np.float32(1.0)

K1 = np.float32(1.0 - 0.1 * float(C1G))            # 1 - 0.1*(1-gamma)
K2 = np.float32(0.1 * float(GAMMA) * float(C1G))   # 0.1*gamma*(1-gamma)
# adaptation state W ~ 0.1*gamma*adapt, updated from u:
#   G_t = g2*G_{t-1} + kk*u_t
KK = np.float32(float(K2) / float(K1))
G2 = np.float32(float(GAMMA) + float(K2) / float(K1))

# per-step geometric scales
A64 = np.float64(ALPHA) ** np.arange(T + 1)      # alpha^t
B64 = np.float64(BETA) ** np.arange(T)           # beta^t
IB64 = 1.0 / B64                                 # input prescale
SV32 = (B64 / A64[:T]).astype(np.float32)        # synv scale immediates
KKA = np.float32(float(KK) / float(ALPHA))
G2A = np.float32(float(G2) / float(ALPHA))

N_PASSES = 1          # 1 = single pass + host margin correction (default)
MARGIN_CAP = 40000    # fall back to a device re-run above this many neurons
THETA0 = 3e-4         # margin radius tracked on device (unscaled u units)

_BUILD_CACHE = {}


# ------------------------------------------------------------- device build
def _build_kernel(n_passes=N_PASSES, base=1.0, bias=0.0):
    key = (n_passes, float(base), float(bias))
    if key in _BUILD_CACHE:
        return _BUILD_CACHE[key]

    import concourse.bacc as bacc
    import concourse.mybir as mybir
    from concourse import tile

    DT = mybir.dt.float32
    AF = mybir.ActivationFunctionType
    OP = mybir.AluOpType
    I32 = mybir.dt.int32

    nc = bacc.Bacc(None, target_bir_lowering=False, debug=False,
                   num_devices=N_CORES)

    x_in = nc.dram_tensor("x", [TB, P, G * FD], DT, kind="ExternalInput")
    # rows: 0 = C' (scaled C), 1 = -C', 2 = -theta0/alpha^t, 3 = 1/alpha^t
    c0_in = nc.dram_tensor("c0", [4, T], DT, kind="ExternalInput")
    # the same rows 0-2 pre-replicated across 128 partitions by the host
    cm_in = nc.dram_tensor("cm", [3, P, T], DT, kind="ExternalInput")

    spk_out = nc.dram_tensor("spk", [TB, P, G * FD], DT, kind="ExternalOutput")
    acc_out = nc.dram_tensor("acc", [3, P, T], DT, kind="ExternalOutput")
    crow_out = nc.dram_tensor("crow", [max(n_passes, 2), T], DT,
                              kind="ExternalOutput")
    xtra_out = nc.dram_tensor("xtra", [1, 8], DT, kind="ExternalOutput")
    mm_out = nc.dram_tensor("mm", [P, FD], DT, kind="ExternalOutput")

    with tile.TileContext(nc) as tc:
        with (
            tc.tile_pool(name="state", bufs=1) as st,
            tc.tile_pool(name="xload", bufs=3) as xl,
            tc.tile_pool(name="sout", bufs=2) as so,
            tc.tile_pool(name="psum", bufs=2, space="PSUM") as ps,
            tc.tile_pool(name="dram", bufs=1, space="DRAM") as dram,
        ):
            mem = st.tile([P, FD], DT, tag="mem")
            gst = st.tile([P, FD], DT, tag="gst")
            mm = st.tile([P, FD], DT, tag="mm")
            spk_s = st.tile([P, FD], DT, tag="spk_s")
            ones = st.tile([P, 1], DT, tag="ones")
            nc.vector.memset(ones[:], 1.0)
            track_margin = n_passes == 1
            if track_margin:
                nc.gpsimd.memset(mm[:], 0.0)

            # broadcast C rows ([128, T]): positive scaled C for the spike
            # compare, negated scaled C for ACT bias, scaled margin radius
            cmat = [
                st.tile([P, T], DT, tag=f"cmat{p}", name=f"cmat{p}")
                for p in range(n_passes)
            ]
            nmat = [
                st.tile([P, T], DT, tag=f"nmat{p}", name=f"nmat{p}")
                for p in range(n_passes)
            ]
            thmat = st.tile([P, T], DT, tag="thmat")
            c0row = st.tile([1, T], DT, tag="c0row")
            ncrow = st.tile([1, T], DT, tag="ncrow")
            throw = st.tile([1, T], DT, tag="throw")
            iarow = st.tile([1, T], DT, tag="iarow")
            nc.sync.dma_start(c0row[:], c0_in[0:1, :])
            nc.sync.dma_start(ncrow[:], c0_in[1:2, :])
            nc.sync.dma_start(throw[:], c0_in[2:3, :])
            nc.sync.dma_start(iarow[:], c0_in[3:4, :])
            nc.sync.dma_start(cmat[0][:], cm_in[0, :, :])
            nc.sync.dma_start(nmat[0][:], cm_in[1, :, :])
            nc.sync.dma_start(thmat[:], cm_in[2, :, :])

            # accumulators: [128, T]: spike counts, u sums, W sums
            acc_spk = st.tile([P, T], DT, tag="acc_spk")
            acc_u = st.tile([P, T], DT, tag="acc_u")
            acc_w = st.tile([P, T], DT, tag="acc_w")

            # constant row of 0.99 for the ema scan (2-pass mode)
            row99 = st.tile([1, T], DT, tag="row99")
            nc.vector.memset(row99[:], 0.99)

            for p_i in range(n_passes):
                last = p_i == n_passes - 1
                C = cmat[p_i]
                NC = nmat[p_i]
                syn = st.tile([P, FD], DT, tag="syn", name="syn0", bufs=2)
                nc.gpsimd.memset(syn[:], 0.0)
                nc.gpsimd.memset(mem[:], 0.0)
                nc.vector.memset(gst[:], 0.0)

                xg = {}

                def load_group(tb_i):
                    if tb_i == 0:
                        xt_a = xl.tile([P, 4 * FD], DT, tag="xta",
                                       name="xta")
                        for q in range(4):
                            nc.sync.dma_start(
                                xt_a[:, q * FD:(q + 1) * FD],
                                x_in[0, :, q * FD:(q + 1) * FD])
                        xt_b = xl.tile([P, 12 * FD], DT, tag="xtb",
                                       name="xtb")
                        nc.sync.dma_start(xt_b[:, 0:6 * FD],
                                          x_in[0, :, 4 * FD:10 * FD])
                        nc.sync.dma_start(xt_b[:, 6 * FD:],
                                          x_in[0, :, 10 * FD:])
                        xg[0] = (xt_a, xt_b)
                    else:
                        xt = xl.tile([P, G * FD], DT, tag="xt", name="xt")
                        nc.sync.dma_start(xt[:], x_in[tb_i, :, :])
                        xg[tb_i] = xt

                def xs_of(t_i):
                    tb_i, g_i = divmod(t_i, G)
                    if tb_i == 0:
                        xt_a, xt_b = xg[0]
                        if g_i < 4:
                            return xt_a[:, g_i * FD:(g_i + 1) * FD]
                        return xt_b[:, (g_i - 4) * FD:(g_i - 3) * FD]
                    return xg[tb_i][:, g_i * FD:(g_i + 1) * FD]

                def syn_step(t_i, syn_prev):
                    # S_t = S_{t-1} + x'_t (GPSIMD, double-buffered);
                    # synv_t = (beta^t/alpha^t) * S_t (ACT)
                    syn_new = st.tile([P, FD], DT, tag="syn",
                                      name="syn", bufs=2)
                    nc.gpsimd.tensor_tensor(syn_new[:], syn_prev[:],
                                            xs_of(t_i), OP.add)
                    synu_n = st.tile([P, FD], DT, tag="synu", name="synu",
                                     bufs=3)
                    nc.scalar.activation(synu_n[:], syn_new[:], AF.Copy,
                                         scale=float(SV32[t_i]))
                    return syn_new, synu_n

                load_group(0)
                syn, synu = syn_step(0, syn)
                ot = None
                for t in range(T):
                    tb, g = divmod(t, G)
                    if g == 0:
                        if tb + 1 < TB:
                            load_group(tb + 1)
                        if last:
                            ot = so.tile([P, G * FD], DT, tag="ot",
                                         name="ot")
                    u = st.tile([P, FD], DT, tag="u", name="u", bufs=2)
                    e = st.tile([P, FD], DT, tag="e", name="e", bufs=2)
                    tmp = st.tile([P, FD], DT, tag="tmp", name="tmp",
                                  bufs=2)
                    ab = st.tile([P, FD], DT, tag="ab", name="ab", bufs=2)
                    sg = st.tile([P, FD], DT, tag="sg", name="sg", bufs=2)
                    # m_t = m'_{t-1} + synv_t (V tensor_tensor: keeps the
                    # reset->membrane->u loop on one engine)
                    nc.vector.tensor_tensor(mem[:], mem[:], synu[:],
                                            OP.add)
                    # u_t = k1*m - W (V stt, accum -> u sums)
                    nc.vector.scalar_tensor_tensor(
                        u[:], mem[:], float(K1), gst[:],
                        OP.mult, OP.subtract,
                        accum_out=acc_u[:, t:t + 1] if last else None,
                    )
                    # reset value e = relu(u + (-C')) (ACT; on the
                    # critical V loop so emitted early)
                    nc.scalar.activation(
                        e[:], u[:], AF.Relu, bias=NC[:, t:t + 1],
                        scale=1.0,
                    )
                    if last:
                        sp_ap = ot[:, g * FD:(g + 1) * FD]
                    else:
                        sp_ap = spk_s[:]
                    # spike = (u >= C'), accum(reduce add) -> step sums
                    nc.vector.tensor_scalar(
                        sp_ap, u[:], C[:, t:t + 1], 0.0,
                        OP.is_ge, OP.add,
                        accum_out=acc_spk[:, t:t + 1],
                    )
                    nc.vector.copy_predicated(
                        mem[:], sp_ap.bitcast(I32), e[:]
                    )
                    # W_t = (g2/a)*W + (kk/a)*u (ACT + V stt, accum)
                    nc.scalar.activation(tmp[:], u[:], AF.Copy,
                                         scale=float(KKA))
                    nc.vector.scalar_tensor_tensor(
                        gst[:], gst[:], float(G2A), tmp[:],
                        OP.mult, OP.add,
                        accum_out=acc_w[:, t:t + 1] if last else None,
                    )
                    # produce next step's synv ahead of the margin ops so
                    # the next membrane add never waits on the ACT queue
                    if t + 1 < T:
                        syn, synu = syn_step(t + 1, syn)
                    if track_margin:
                        nc.scalar.activation(
                            ab[:], u[:], AF.Abs, bias=NC[:, t:t + 1],
                            scale=1.0,
                        )
                        nc.scalar.activation(
                            sg[:], ab[:], AF.Sign,
                            bias=thmat[:, t:t + 1], scale=1.0,
                        )
                        nc.gpsimd.tensor_tensor(mm[:], mm[:], sg[:],
                                                OP.add)
                    if last and g == G - 1:
                        nc.sync.dma_start(spk_out[tb, :, :], ot[:])

                # ---- inter-pass: global spike sums -> next C (2-pass mode)
                if not last:
                    pt = ps.tile([1, T], DT, tag="pt")
                    nc.tensor.matmul(pt[:], ones[:], acc_spk[:],
                                     start=True, stop=True)
                    srow = st.tile([1, T], DT, tag=f"srow{p_i}",
                                   name=f"srow{p_i}")
                    nc.scalar.copy(srow[:], pt[:])
                    cc_in = dram.tile([1, T], DT, name=f"ccin{p_i}")
                    cc_out = dram.tile([1, T], DT, name=f"ccout{p_i}")
                    nc.sync.dma_start(cc_in[:], srow[:])
                    nc.gpsimd.collective_compute(
                        "AllReduce", OP.add,
                        replica_groups=[list(range(N_CORES))],
                        ins=[cc_in.opt()], outs=[cc_out.opt()],
                    )
                    sglob = st.tile([1, T], DT, tag=f"sglob{p_i}",
                                    name=f"sglob{p_i}")
                    nc.sync.dma_start(sglob[:], cc_out[:])
                    # ema chain: msc = sums * (0.01/N_GLOB); scan; homeo; C
                    mean_scale = np.float32(
                        np.float32(0.01) * np.float32(1.0 / N_GLOB))
                    msc = st.tile([1, T], DT, tag=f"msc{p_i}",
                                  name=f"msc{p_i}")
                    nc.vector.tensor_scalar(
                        msc[:], sglob[:], float(mean_scale), None, OP.mult)
                    ema = st.tile([1, T], DT, tag=f"ema{p_i}",
                                  name=f"ema{p_i}")
                    nc.vector.tensor_tensor_scan(
                        ema[:], row99[:], msc[:], 0.0, OP.mult, OP.add)
                    hm = st.tile([1, T], DT, tag=f"hm{p_i}", name=f"hm{p_i}")
                    nc.vector.tensor_scalar(
                        hm[:], ema[:], float(HOMEO_RATE), 0.0,
                        OP.subtract, OP.max)
                    cn = st.tile([1, T], DT, tag=f"cn{p_i}", name=f"cn{p_i}")
                    # C[0] is pass-independent (ema starts at 0; scale=1)
                    nc.vector.tensor_copy(cn[:, 0:1], c0row[:, 0:1])
                    nc.vector.tensor_scalar(
                        cn[:, 1:T], hm[:, 0:T - 1], float(ADAPT_STRENGTH),
                        float(base), OP.mult, OP.add)
                    if float(bias) != 0.0:
                        nc.vector.tensor_scalar(
                            cn[:, 1:T], cn[:, 1:T], float(bias), None,
                            OP.add)
                    # scale to C' = C/alpha^t (input row 3 = 1/alpha^t)
                    csc = st.tile([1, T], DT, tag=f"csc{p_i}",
                                  name=f"csc{p_i}")
                    nc.vector.tensor_tensor(csc[:], cn[:], iarow[:],
                                            OP.mult)
                    ncn = st.tile([1, T], DT, tag=f"ncn{p_i}",
                                  name=f"ncn{p_i}")
                    nc.vector.tensor_scalar(
                        ncn[:], csc[:], -1.0, None, OP.mult)
                    nc.gpsimd.partition_broadcast(
                        cmat[p_i + 1][:], csc[:], channels=P)
                    nc.gpsimd.partition_broadcast(
                        nmat[p_i + 1][:], ncn[:], channels=P)
                    nc.sync.dma_start(crow_out[p_i + 1:p_i + 2, :], cn[:])

            # ---- final extras: sum of post-reset scaled mem at t=T-1
            memsum = st.tile([P, 1], DT, tag="memsum")
            nc.vector.reduce_sum(memsum[:], mem[:], axis=mybir.AxisListType.X)
            ptx = ps.tile([1, 1], DT, tag="ptx")
            nc.tensor.matmul(ptx[:], ones[:], memsum[:], start=True, stop=True)
            xrow = st.tile([1, 8], DT, tag="xrow")
            nc.vector.memset(xrow[:], 0.0)
            nc.scalar.copy(xrow[:, 0:1], ptx[:])
            nc.sync.dma_start(xtra_out[:, :], xrow[:])

            # C' used in pass 0 (echo back for debugging)
            nc.sync.dma_start(crow_out[0:1, :], c0row[:, :])

            # accumulators + margin counters out
            nc.sync.dma_start(acc_out[0, :, :], acc_spk[:])
            nc.sync.dma_start(acc_out[1, :, :], acc_u[:])
            nc.sync.dma_start(acc_out[2, :, :], acc_w[:])
            if track_margin:
                nc.sync.dma_start(mm_out[:, :], mm[:])
            else:
                nc.sync.dma_start(mm_out[:, :], spk_s[:])

    nc.compile()
    _BUILD_CACHE[key] = nc
    return nc


# --------------------------------------------------------------- host logic
def _ema_chain_from_sums(sums, base, bias):
    """Replicate the reference's scalar f32 EMA/homeo chain from global
    per-step spike sums (exact integers).  Returns (C[t] used AT step t,
    final ema)."""
    ema = np.float32(0.0)
    C = np.zeros(T, np.float32)
    for t in range(T):
        homeo = np.float32(ADAPT_STRENGTH * max(np.float32(ema - HOMEO_RATE),
                                                np.float32(0.0)))
        C[t] = np.float32(np.float32(base + homeo) + bias)
        mean = np.float32(np.float32(sums[t]) * np.float32(1.0 / N_GLOB))
        ema = np.float32(math.fma(float(np.float32(0.99)), float(ema),
                                  float(np.float32(np.float32(0.01) * mean))))
    return C, ema


def _estimate_c0(input_current, base, bias, stride=16):
    """Cheap numpy f32 simulation of a strided neuron subsample with a local
    EMA to estimate the C[t] threshold-offset sequence."""
    x = np.ascontiguousarray(
        input_current[:, ::stride, :], dtype=np.float32)  # [B, F/stride, T]
    nb, nf, nt = x.shape
    x = x.reshape(nb * nf, nt)
    syn = np.zeros(nb * nf, np.float32)
    mem = np.zeros(nb * nf, np.float32)
    adp = np.zeros(nb * nf, np.float32)
    ema = np.float32(0.0)
    C = np.zeros(nt, np.float32)
    for t in range(nt):
        syn = BETA * syn + x[:, t]
        mem = ALPHA * mem + syn
        homeo = np.float32(ADAPT_STRENGTH * max(np.float32(ema - HOMEO_RATE),
                                                np.float32(0.0)))
        adp = GAMMA * adp + C1G * mem
        C[t] = np.float32(np.float32(base + homeo) + bias)
        thr = C[t] + ADAPT_STRENGTH * adp
        spike = (mem >= thr)
        ema = np.float32(0.99 * ema + 0.01 * np.float32(spike.mean()))
        mem = np.where(spike, mem - thr, mem).astype(np.float32)
    return C


def _chat_rows(C):
    """Device input rows for a given (unscaled) C chain: [C', -C',
    -theta0/alpha^t, 1/alpha^t] as [4, T], plus the same first three rows
    replicated across 128 partitions as [3, P, T]."""
    chat = (C.astype(np.float64) / A64[:T]).astype(np.float32)
    th = (np.float64(THETA0) / A64[:T]).astype(np.float32)
    ia = (1.0 / A64[:T]).astype(np.float32)
    rows = np.stack([chat, -chat, -th, ia]).astype(np.float32)
    cm = np.ascontiguousarray(
        np.broadcast_to(rows[:3, None, :], (3, P, T)), dtype=np.float32)
    return (rows, cm), chat


def _sim_neurons(x_rows, chat):
    """Bit-exact numpy replica of the device per-neuron chain (scaled
    coordinates).  x_rows: [n, T] f32 prescaled inputs.  chat: [T] f32
    scaled C' sequence.  Returns per-step spikes/u/W plus final m'."""
    n = x_rows.shape[0]
    syn = np.zeros(n, np.float32)
    mem = np.zeros(n, np.float32)
    gst = np.zeros(n, np.float32)
    spikes = np.zeros((n, T), np.float32)
    u_tr = np.zeros((n, T), np.float32)
    w_tr = np.zeros((n, T), np.float32)
    for t in range(T):
        syn = (syn + x_rows[:, t]).astype(np.float32)
        synv = (syn * SV32[t]).astype(np.float32)
        mem = (mem + synv).astype(np.float32)
        u = ((mem * K1).astype(np.float32) - gst).astype(np.float32)
        u_tr[:, t] = u
        sp = u >= chat[t]
        spikes[:, t] = sp
        ev = np.maximum((u - chat[t]).astype(np.float32), np.float32(0.0))
        mem = np.where(sp, ev, mem).astype(np.float32)
        tmp = (u * KKA).astype(np.float32)
        gst = ((gst * G2A).astype(np.float32) + tmp).astype(np.float32)
        w_tr[:, t] = gst
    return {"spikes": spikes, "u": u_tr, "w": w_tr, "m_last": mem}


def _prescale(input_current):
    """x'_[b,f,t] = f32(x / beta^t) plus its per-step global f64 sums."""
    x64 = input_current.astype(np.float64) * IB64[None, None, :]
    xp = x64.astype(np.float32)
    sxp = x64.sum(axis=(0, 1))
    return xp, sxp


def _shard_input(xp):
    """prescaled [B, F, T] -> per-core [TB, P, G*FD] time-major tiles."""
    xs = []
    for k in range(N_CORES):
        xk = xp[:, k * F_LOC:(k + 1) * F_LOC, :]  # [B, F_LOC, T]
        xk = np.ascontiguousarray(np.moveaxis(xk, -1, 0)).reshape(T, N_LOC)
        xk = xk.reshape(TB, G, P, FD).transpose(0, 2, 1, 3)
        xs.append(np.ascontiguousarray(xk.reshape(TB, P, G * FD),
                                       dtype=np.float32))
    return xs


def _unshard_spikes(spk_cores):
    """per-core [TB, P, G*FD] -> [B, F, T] float32."""
    out = np.empty((B, F, T), np.float32)
    for k, sk in enumerate(spk_cores):
        sk = sk.reshape(TB, P, G, FD).transpose(0, 2, 1, 3).reshape(T, N_LOC)
        sk = np.moveaxis(sk.reshape(T, B, F_LOC), 0, -1)
        out[:, k * F_LOC:(k + 1) * F_LOC, :] = sk
    return out


def _run_device(xs, crows, base, bias, n_passes):
    from concourse.bass_utils import run_bass_kernel_spmd

    rows, cm = crows
    nc = _build_kernel(n_passes, float(base), float(bias))
    in_maps = [{"x": xs[k], "c0": rows, "cm": cm} for k in range(N_CORES)]
    res = run_bass_kernel_spmd(nc, in_maps, core_ids=list(range(N_CORES)))
    return res.results


def _unscale_sums(s_u, s_w, xtra_sum):
    """Scaled per-step u/W sums -> unscaled mem_pre / G sums (f64)."""
    s_w_prev = np.concatenate([[0.0], s_w[:-1]])
    s_mem = A64[:T] * (s_u + s_w_prev) / np.float64(K1)
    s_g = A64[1:T + 1] * s_w
    mlast = xtra_sum * A64[T - 1]
    return s_mem, s_g, mlast


def kernel(input_current, threshold_scale, adaptation_bias):
    input_current = np.asarray(input_current, np.float32)
    threshold_scale = np.asarray(threshold_scale, np.float32)
    adaptation_bias = np.asarray(adaptation_bias, np.float32)
    assert input_current.shape == (B, F, T)

    scale_v = np.float32(threshold_scale.reshape(-1)[0])
    bias_v = np.float32(adaptation_bias.reshape(-1)[0])
    base_v = np.float32(THRESHOLD_BASE * scale_v)

    c0 = _estimate_c0(input_current, base_v, bias_v)
    xp, sxp = _prescale(input_current)
    xs = _shard_input(xp)

    if N_PASSES != 1:
        return _kernel_multipass(xs, sxp, c0, base_v, bias_v)

    for attempt in range(3):
        crows, chat0 = _chat_rows(c0)
        r = _run_device(xs, crows, base_v, bias_v, 1)
        out = _finish_single(xp, sxp, c0, chat0, base_v, bias_v, r)
        if out is not None:
            return out
        # margin set too large -> c0 was far off; retry with corrected C
        acc = np.stack([rk["acc"] for rk in r])
        s_spk = acc.astype(np.float64).sum(axis=(0, 2))[0]
        c0, _ = _ema_chain_from_sums(s_spk, base_v, bias_v)
    # last resort: fully on-device 2-pass mode
    return _kernel_multipass(xs, sxp, c0, base_v, bias_v)


def _finish_single(xp, sxp, c0, chat0, base_v, bias_v, r):
    acc = np.stack([rk["acc"] for rk in r])       # [8, 3, P, T]
    g_sums = acc.astype(np.float64).sum(axis=(0, 2))
    s_spk, s_u, s_w = g_sums
    mm = np.stack([rk["mm"] for rk in r])          # [8, P, FD]
    mm_flat = mm.reshape(N_CORES, N_LOC)           # neuron n = p*FD + j

    # candidate corrected C chain
    C1, _ = _ema_chain_from_sums(s_spk, base_v, bias_v)
    delta = float(np.abs(C1.astype(np.float64) - c0.astype(np.float64)).max())
    theta = 6.0 * delta + 1e-6
    if theta > THETA0:
        return None  # tracked margin radius insufficient; retry

    sel = mm_flat < np.float32(T)                  # [8, N_LOC] bool
    n_sel = int(sel.sum())
    if n_sel > MARGIN_CAP:
        return None

    # gather prescaled x rows for selected neurons
    core_idx, loc_idx = np.nonzero(sel)
    p_idx = loc_idx // FD
    j_idx = loc_idx % FD
    b_idx = loc_idx // F_LOC
    f_idx = core_idx * F_LOC + (loc_idx % F_LOC)
    x_rows = np.ascontiguousarray(xp[b_idx, f_idx, :], dtype=np.float32)

    # device-replica sim under c0: must reproduce the device bitwise
    sim0 = _sim_neurons(x_rows, chat0)

    spk_cores = [r[k]["spk"] for k in range(N_CORES)]
    tbv, gv = np.divmod(np.arange(T), G)
    dev_rows = np.zeros((n_sel, T), np.float32)
    for i in range(n_sel):
        sk = spk_cores[core_idx[i]]
        dev_rows[i] = sk[tbv, p_idx[i], gv * FD + j_idx[i]]
    mismatch = int((dev_rows != sim0["spikes"]).sum())
    if mismatch:
        sys.stderr.write(
            f"kernel.py: host/device replica mismatch ({mismatch} elems), "
            "falling back\n")
        return None

    # iterate the tiny fixed point on the selected set (scaled sums)
    base_spk = s_spk - sim0["spikes"].astype(np.float64).sum(axis=0)
    base_u = s_u - sim0["u"].astype(np.float64).sum(axis=0)
    base_w = s_w - sim0["w"].astype(np.float64).sum(axis=0)
    xtra0 = float(sum(rk["xtra"][0, 0] for rk in r))
    base_mlast = xtra0 - float(sim0["m_last"].astype(np.float64).sum())

    C_cur = C1
    sim = sim0
    converged = False
    for _ in range(12):
        chat_cur = (C_cur.astype(np.float64) / A64[:T]).astype(np.float32)
        sim = _sim_neurons(x_rows, chat_cur)
        s_spk_new = base_spk + sim["spikes"].astype(np.float64).sum(axis=0)
        C_next, _ = _ema_chain_from_sums(s_spk_new, base_v, bias_v)
        if np.array_equal(C_next.view(np.uint32), C_cur.view(np.uint32)):
            converged = True
            break
        C_cur = C_next
    if not converged:
        return None

    # safety: all excursions must stay inside the tracked margin radius
    exc = float(np.abs(C_cur.astype(np.float64) -
                       c0.astype(np.float64)).max())
    if exc > theta / 2.0:
        return None

    s_spk = base_spk + sim["spikes"].astype(np.float64).sum(axis=0)
    s_u = base_u + sim["u"].astype(np.float64).sum(axis=0)
    s_w = base_w + sim["w"].astype(np.float64).sum(axis=0)
    mlast_s = base_mlast + float(sim["m_last"].astype(np.float64).sum())
    _, ema_final = _ema_chain_from_sums(s_spk, base_v, bias_v)

    spikes = _unshard_spikes(spk_cores)
    spikes[b_idx, f_idx, :] = sim["spikes"]

    s_mem, s_g, mlast = _unscale_sums(s_u, s_w, mlast_s)
    return _assemble(sxp, spikes, s_mem, s_g, mlast, C_cur, ema_final)


def _assemble(sxp, spikes, s_mem, s_g, mem_last_sum, C_used, ema_final):
    # syn sums reconstructed from prescaled-input sums (f64 model):
    # S_t = S_{t-1} + sum(x'_t);  syn_t = beta^t * S_t
    s_syn = np.zeros(T, np.float64)
    run = 0.0
    for t in range(T):
        run = run + sxp[t]
        s_syn[t] = B64[t] * run

    mem_post = np.empty(T, np.float64)
    mem_post[:T - 1] = (s_mem[1:] - s_syn[1:]) / np.float64(ALPHA)
    mem_post[T - 1] = mem_last_sum
    mem_trace = (mem_post / N_GLOB).astype(np.float32)

    thr_trace = (C_used.astype(np.float64)
                 + s_g / (np.float64(GAMMA) * N_GLOB)).astype(np.float32)

    adaptation_mean = np.float32(
        s_g[T - 1] / (0.1 * np.float64(GAMMA)) / N_GLOB)

    return (spikes, mem_trace, thr_trace, np.float32(ema_final),
            adaptation_mean)


def _kernel_multipass(xs, sxp, c0, base_v, bias_v):
    crows, _ = _chat_rows(c0)
    r = _run_device(xs, crows, base_v, bias_v, 2)
    acc = np.stack([rk["acc"] for rk in r])
    g_sums = acc.astype(np.float64).sum(axis=(0, 2))
    s_spk, s_u, s_w = g_sums
    c_dev = r[0]["crow"][1].astype(np.float32)
    _, ema_final = _ema_chain_from_sums(s_spk, base_v, bias_v)
    xtra_sum = float(sum(rk["xtra"][0, 0] for rk in r))
    spikes = _unshard_spikes([rk["spk"] for rk in r])
    s_mem, s_g, mlast = _unscale_sums(s_u, s_w, xtra_sum)
    return _assemble(sxp, spikes, s_mem, s_g, mlast, c_dev, ema_final)


if __name__ == "__main__":
    rng = np.random.default_rng(0)
    x = rng.standard_normal((B, F, T)).astype(np.float32)
    t0 = time.time()
    out = kernel(x, np.ones(1, np.float32), np.zeros(1, np.float32))
    print("kernel wall:", time.time() - t0)
    for o in out:
        print(np.shape(o), np.asarray(o).dtype)
